# revision 14
# baseline (speedup 1.0000x reference)
"""Trainium2 Bass kernel for nn_AxonalConnections.

Computes, per (batch b, patch n):
    out[t]  = sum_s sp[b,n,s] * W_dyn[b,n,t,s]          (batched matvec, distinct weights)
    out_n   = LayerNorm_T(out) * gamma + beta
    w       = softmax(out_n / TEMP)
    final   = w * (gates[n] * sum_s sp[b,n,s] + biases[n])
    fold -> [B, 256, 256]

Strategy: 8-way shard over (batch b, patch-half); each core owns 128 patches.
Spikes are binary with ~0.1 density, so out[t] is just the SUM of the ~26
active columns W[:, s] per patch.  The host gathers only those columns
(~10% of W), packs them into a flat ragged layout [K, T] (patch-major),
and ships them split as bf16 hi + fp8e4m3 lo (combined ~2^-13 relative
error).  The device reduces each patch's segment with the TensorEngine:
    psum[n, t] = sum_k M[k, n] * C_hi[k, t]  +  Mlo[k, n] * C_lo[k, t]
where M is the one-hot patch-membership matrix (generated on-device from a
tiny patch-id vector: M[k, n] = (pid[k] == n)) and Mlo = M * 2^-LOSH folds
the lo-residual scale into the lhsT so hi and lo accumulate into the same
PSUM region.  K is padded to a multiple of 128; each 128-column chunk is
one matmul per hi/lo pass.

Latency shaping: patches are split into two 64-row halves packed into
disjoint chunk ranges, so half A's PSUM closes mid-stream and its
LayerNorm+softmax epilogue overlaps half B's matmuls; only half B's
epilogue sits in the serial tail.  Dummy matmuls during the initial DMA
latency window pre-ramp the PE clock, and both activation tables
(Sqrt, Exp+accum) are warmed up front so no table load lands in the tail.
HBM traffic per core is ~3.1MB vs ~25.7MB for the dense hi/lo kernel.
"""

import sys

for _p in ("/opt/trn_rl_repo",):
    if _p not in sys.path:
        sys.path.insert(0, _p)

import numpy as np
import ml_dtypes

import concourse.bass as bass
import concourse.bacc as bacc
import concourse.tile as tile
from concourse import mybir
from concourse import bass_utils

# Problem constants (hardcoded per contract)
B = 4
GRID = 256
PATCH = 16
PH = GRID // PATCH          # 16 patches per side
N = PH * PH                 # 256 patches
S = PATCH * PATCH           # 256 source pixels per patch
T = 256                     # 256 target pixels per patch
TEMP = 0.1
LN_EPS = 1e-5

NCORES = 8
P = 128                     # patches per core (= SBUF partitions)
H = 64                      # patches per epilogue half
GC = 4                      # chunks per DMA group
MAX_NCH = 16                # cap per half: 2048 slots (mean 1638, sigma 38)
LOSH = 12                   # lo residual shipped as fp8e4m3 scaled by 2**LOSH
NWARM = 6                   # PE clock pre-ramp matmuls

F32 = mybir.dt.float32
BF16 = mybir.dt.bfloat16
NP_BF16 = ml_dtypes.bfloat16
NP_FP8 = ml_dtypes.float8_e4m3

_NC_CACHE = {}


def _groups_of(nch):
    g = [GC] * (nch // GC)
    if nch % GC:
        g.append(nch % GC)
    return g


def _build_nc(nch_a, nch_b):
    """Bass program for one core.  Chunks [0, nch_a) hold patch 0-63
    columns, [nch_a, nch_a+nch_b) hold patch 64-127 columns."""
    nc = bacc.Bacc("TRN2")
    nch = nch_a + nch_b
    chi = nc.dram_tensor("chi", [P, nch * T], BF16, kind="ExternalInput")
    clo = nc.dram_tensor("clo", [P, nch * T], mybir.dt.float8e4,
                         kind="ExternalInput")
    # patch id owning each column slot (-1 for padding): [k%128, chunk]
    pid = nc.dram_tensor("pid", [P, nch], BF16, kind="ExternalInput")
    # iota over patch index n (same row 0..127 in every partition)
    iot = nc.dram_tensor("iot", [P, P], BF16, kind="ExternalInput")
    sp = nc.dram_tensor("sp", [P, S], F32, kind="ExternalInput")
    # packed per-core params: [gamma/TEMP (256) | beta/TEMP (256) | gate | bias]
    prm = nc.dram_tensor("prm", [P, 2 * T + 2], F32, kind="ExternalInput")
    outd = nc.dram_tensor("out", [P, T], F32, kind="ExternalOutput")

    Alu = mybir.AluOpType
    Act = mybir.ActivationFunctionType
    Ax = mybir.AxisListType

    # (start_chunk, group_size, half) list
    sched = []
    for half, (base, cnt) in enumerate(((0, nch_a), (nch_a, nch_b))):
        c0 = base
        for gp in _groups_of(cnt):
            sched.append((c0, gp, half))
            c0 += gp

    with tile.TileContext(nc) as tc:
        with (
            tc.tile_pool(name="wpool", bufs=4 * len(sched)) as wpool,
            tc.tile_pool(name="pspool", bufs=1, space="PSUM") as pspool,
            tc.tile_pool(name="sing", bufs=1) as sing,
            tc.tile_pool(name="small", bufs=4) as small,
        ):
            # tiny tensors first: pid/iot gate the membership matrices the
            # first matmul needs
            pid_t = sing.tile([P, nch], BF16)
            nc.sync.dma_start(out=pid_t, in_=pid[:, :])
            iot_t = sing.tile([P, P], BF16)
            nc.scalar.dma_start(out=iot_t, in_=iot[:, :])

            # column stream: group DMAs on two queues; sp/prm ride after the
            # first group (only needed by the epilogue)
            for gi, (c0, gp, half) in enumerate(sched):
                chit = wpool.tile([P, gp, T], BF16, tag="chit")
                nc.sync.dma_start(
                    out=chit.rearrange("p c t -> p (c t)"),
                    in_=chi[:, c0 * T : (c0 + gp) * T])
                clot = wpool.tile([P, gp, T], mybir.dt.float8e4, tag="clot")
                nc.scalar.dma_start(
                    out=clot.rearrange("p c t -> p (c t)"),
                    in_=clo[:, c0 * T : (c0 + gp) * T])
                sched[gi] = (c0, gp, half, chit, clot)
                if gi == 0:
                    sp_t = sing.tile([P, S], F32)
                    nc.scalar.dma_start(out=sp_t, in_=sp[:, :])
                    prm_t = sing.tile([P, 2 * T + 2], F32)
                    nc.scalar.dma_start(out=prm_t, in_=prm[:, :])

            gmt_t = prm_t[:, 0:T]
            bft_t = prm_t[:, T : 2 * T]
            gat_t = prm_t[:, 2 * T : 2 * T + 1]
            bia_t = prm_t[:, 2 * T + 1 : 2 * T + 2]

            # ---- PE clock pre-ramp on scratch data (no DMA dependency) ----
            scr = sing.tile([P, T], BF16)
            nc.vector.memset(scr, 0.0)
            ps_warm = pspool.tile([P, T], F32)
            for _ in range(NWARM):
                nc.tensor.matmul(ps_warm, lhsT=scr[:, 0:P], rhs=scr,
                                 start=True, stop=True)

            eps_t = small.tile([P, 1], F32)
            nc.vector.memset(eps_t, LN_EPS)
            # warm BOTH activation tables with the exact variants used in the
            # epilogue (Sqrt w/ bias AP; Exp w/ bias AP + accum) so no
            # ~1.3us ACT_TABLE_LOAD lands in the serial tail
            w1 = small.tile([P, 1], F32)
            nc.scalar.activation(out=w1, in_=eps_t, func=Act.Sqrt,
                                 bias=eps_t, scale=1.0)
            w2 = small.tile([P, 1], F32)
            w2d = small.tile([P, 1], F32)
            nc.scalar.activation(out=w2, in_=eps_t, func=Act.Exp,
                                 bias=eps_t, scale=1.0, accum_out=w2d)

            # ---- membership matrices M[p, c, n], Mlo = M * 2^-LOSH ----
            # absorb the pid/iot DMA waits into non-TT DVE ops first
            # (plain TensorTensor only survives walrus codegen with <=1 wait)
            tch1 = small.tile([P, 1], F32)
            nc.vector.tensor_scalar_mul(tch1, pid_t[:, 0:1], 1.0)
            tch2 = small.tile([P, 1], F32)
            nc.vector.tensor_scalar_mul(tch2, iot_t[:, 0:1], 1.0)
            for gi, (c0, gp, half, chit, clot) in enumerate(sched):
                mg = wpool.tile([P, gp, H], BF16, tag="mg")
                nc.vector.tensor_tensor(
                    out=mg,
                    in0=pid_t[:, c0 : c0 + gp].unsqueeze(2)
                        .broadcast_to((P, gp, H)),
                    in1=iot_t[:, half * H : (half + 1) * H].unsqueeze(1)
                        .broadcast_to((P, gp, H)),
                    op=Alu.is_equal)
                mglo = wpool.tile([P, gp, H], BF16, tag="mglo")
                nc.vector.tensor_scalar_mul(mglo, mg, float(2.0 ** -LOSH))
                sched[gi] = (c0, gp, half, chit, clot, mg, mglo)

            # per-patch scalar chain: gates * sum_s(sp) + biases (early,
            # full 128 rows at once; sliced per half at use)
            spsum = small.tile([P, 1], F32)
            nc.vector.tensor_reduce(out=spsum, in_=sp_t, axis=Ax.X, op=Alu.add)
            scal = small.tile([P, 1], F32)
            nc.vector.tensor_mul(scal, gat_t, spsum)
            scal2 = small.tile([P, 1], F32)
            nc.vector.tensor_add(scal2, scal, bia_t)

            # ---- segmented sum via PE: ps[n,t] = sum_k M[k,n] C[k,t] ----
            # hi and lo passes accumulate into the same PSUM region (the
            # 2^-LOSH scale is folded into Mlo); separate PSUM tiles per
            # half so half A's readiness is tracked independently
            ps_a = pspool.tile([P, T], F32)
            ps_b = pspool.tile([P, T], F32)
            ps_h = [ps_a, ps_b]
            for c0, gp, half, chit, clot, mg, mglo in sched:
                r = slice(half * H, (half + 1) * H)
                ps = ps_h[half]
                first = {0: 0, 1: nch_a}[half]
                last = {0: nch_a - 1, 1: nch - 1}[half]
                for j in range(gp):
                    c = c0 + j
                    nc.tensor.matmul(
                        ps[r, :], lhsT=mg[:, j, :], rhs=chit[:, j, :],
                        start=(c == first), stop=False)
                    nc.tensor.matmul(
                        ps[r, :], lhsT=mglo[:, j, :], rhs=clot[:, j, :],
                        start=False, stop=(c == last))

            # ---- per-half epilogue: LayerNorm + temperature softmax ----
            # half A's chain overlaps half B's matmul stream
            fin = small.tile([P, T], F32)
            for half in range(2):
                r = slice(half * H, (half + 1) * H)
                ps = ps_h[half]
                stats = small.tile([P, 6], F32, tag=f"stats{half}")
                nc.vector.bn_stats(out=stats[r, :], in_=ps[r, :])
                mv = small.tile([P, 2], F32, tag=f"mv{half}")
                nc.vector.bn_aggr(out=mv[r, :], in_=stats[r, :])
                std = small.tile([P, 1], F32, tag=f"std{half}")
                nc.scalar.activation(out=std[r, :], in_=mv[r, 1:2],
                                     func=Act.Sqrt, bias=eps_t[r, :],
                                     scale=1.0)
                rstd = small.tile([P, 1], F32, tag=f"rstd{half}")
                nc.vector.reciprocal(out=rstd[r, :], in_=std[r, :])
                z1 = small.tile([P, T], F32, tag=f"z1{half}")
                nc.vector.tensor_scalar(out=z1[r, :], in0=ps[r, :],
                                        scalar1=mv[r, 0:1],
                                        scalar2=rstd[r, :],
                                        op0=Alu.subtract, op1=Alu.mult)
                z2 = small.tile([P, T], F32, tag=f"z2{half}")
                nc.vector.tensor_mul(z2[r, :], z1[r, :], gmt_t[r, :])
                z3 = small.tile([P, T], F32, tag=f"z3{half}")
                nc.vector.tensor_add(z3[r, :], z2[r, :], bft_t[r, :])

                negmx = small.tile([P, 1], F32, tag=f"negmx{half}")
                nc.vector.tensor_reduce(out=negmx[r, :], in_=z3[r, :],
                                        axis=Ax.X, op=Alu.max, negate=True)
                e = small.tile([P, T], F32, tag=f"e{half}")
                den = small.tile([P, 1], F32, tag=f"den{half}")
                nc.scalar.activation(out=e[r, :], in_=z3[r, :], func=Act.Exp,
                                     bias=negmx[r, :], scale=1.0,
                                     accum_out=den[r, :])
                rden = small.tile([P, 1], F32, tag=f"rden{half}")
                nc.vector.reciprocal(out=rden[r, :], in_=den[r, :])
                fac = small.tile([P, 1], F32, tag=f"fac{half}")
                nc.vector.tensor_mul(fac[r, :], scal2[r, :], rden[r, :])
                nc.vector.tensor_scalar_mul(fin[r, :], e[r, :], fac[r, :])
                nc.sync.dma_start(out=outd[half * H : (half + 1) * H, :],
                                  in_=fin[r, :])
    nc.compile()
    return nc


def _get_nc(key=None):
    if key is None:
        key = _NC_CACHE.get("last_key", (13, 13))
    if key not in _NC_CACHE:
        _NC_CACHE[key] = _build_nc(*key)
    return _NC_CACHE[key]


def _to_bf16_bits(x):
    # round-to-nearest-even bf16 via uint bit trick (ml_dtypes astype is
    # far too slow for MB-scale arrays)
    u = x.view(np.uint32)
    rounded = u + 0x7FFF + ((u >> 16) & 1)
    return (rounded >> 16).astype(np.uint16)


def _to_e4m3(x):
    # fast fp8e4m3 RNE for |x| < 448, with subnormals
    u = x.view(np.uint32)
    s = ((u >> 24) & 0x80).astype(np.uint32)
    mag = u & 0x7FFFFFFF
    r = mag + 0x7FFFF + ((mag >> 20) & 1)
    exp = (r >> 23).astype(np.int32) - 120      # e4m3-biased exponent
    man = (r >> 20) & 0x7
    # subnormal path: round(|x| * 2^9) gives the denormal bits directly
    man_d = np.rint(np.abs(x) * 512.0).astype(np.uint32)
    out = np.where(exp >= 1, (exp.astype(np.uint32) << 3) | man, man_d)
    return (s | out).astype(np.uint8)


def _make_in_maps(source_spikes, W_dyn, ln_gamma, ln_beta, gates, biases):
    source_spikes = np.asarray(source_spikes, dtype=np.float32)
    W_dyn = np.asarray(W_dyn, dtype=np.float32)
    ln_gamma = np.asarray(ln_gamma, dtype=np.float32)
    ln_beta = np.asarray(ln_beta, dtype=np.float32)
    gates = np.asarray(gates, dtype=np.float32)
    biases = np.asarray(biases, dtype=np.float32)

    # unfold (matches reference._unfold with kernel=stride=16)
    sp_unf = (
        source_spikes.reshape(B, PH, PATCH, PH, PATCH)
        .transpose(0, 1, 3, 2, 4)
        .reshape(B, N, S)
    )
    sp_unf = np.ascontiguousarray(sp_unf)

    # active-column index lists per core (patch-major order), split at the
    # patch-64 boundary so each half occupies its own chunk range
    cores = []
    nch_a = nch_b = 1
    for c in range(NCORES):
        b, h = divmod(c, NCORES // B)
        n0 = h * P
        spv = np.ascontiguousarray(sp_unf[b, n0 : n0 + P])
        pid_arr, s_arr = np.nonzero(spv)
        ka = int(np.searchsorted(pid_arr, H))
        cores.append((b, n0, spv, pid_arr, s_arr, ka))
        nch_a = max(nch_a, -(-ka // P))
        nch_b = max(nch_b, -(-(len(pid_arr) - ka) // P))
    assert nch_a <= MAX_NCH and nch_b <= MAX_NCH, (
        f"active-column overflow: {nch_a}/{nch_b} chunks > {MAX_NCH}")
    _NC_CACHE["last_key"] = (nch_a, nch_b)
    nch = nch_a + nch_b

    iot = np.ascontiguousarray(np.broadcast_to(
        np.arange(P, dtype=np.float32).astype(NP_BF16), (P, P)))

    in_maps = []
    for b, n0, spv, pid_arr, s_arr, ka in cores:
        k = len(pid_arr)
        # gather active columns W_dyn[b, n0+pid, :, s] -> [k, T]
        cols = W_dyn[b, n0 : n0 + P][pid_arr, :, s_arr]
        hi_bits = _to_bf16_bits(cols)
        hi_f32 = (hi_bits.astype(np.uint32) << 16).view(np.float32)
        lo_bits = _to_e4m3((cols - hi_f32) * float(2 ** LOSH))

        hi_pad = np.zeros((nch * P, T), dtype=np.uint16)
        lo_pad = np.zeros((nch * P, T), dtype=np.uint8)
        pid_pad = np.full(nch * P, -1.0, dtype=np.float32)
        hi_pad[:ka] = hi_bits[:ka]
        lo_pad[:ka] = lo_bits[:ka]
        pid_pad[:ka] = pid_arr[:ka]
        hi_pad[nch_a * P : nch_a * P + (k - ka)] = hi_bits[ka:]
        lo_pad[nch_a * P : nch_a * P + (k - ka)] = lo_bits[ka:]
        pid_pad[nch_a * P : nch_a * P + (k - ka)] = pid_arr[ka:]

        def pack(flat):
            v = flat.reshape(nch, P, T).transpose(1, 0, 2)
            return np.ascontiguousarray(v.reshape(P, nch * T))

        prm = np.empty((P, 2 * T + 2), dtype=np.float32)
        prm[:, 0:T] = ln_gamma / TEMP
        prm[:, T : 2 * T] = ln_beta / TEMP
        prm[:, 2 * T] = gates[n0 : n0 + P]
        prm[:, 2 * T + 1] = biases[n0 : n0 + P]

        in_maps.append({
            "chi": pack(hi_pad).view(NP_BF16),
            "clo": pack(lo_pad).view(NP_FP8),
            "pid": np.ascontiguousarray(
                pid_pad.reshape(nch, P).T.astype(NP_BF16)),
            "iot": iot,
            "sp": spv,
            "prm": prm,
        })
    return in_maps


def _assemble(results):
    out_bnt = np.empty((B, N, T), dtype=np.float32)
    for c in range(NCORES):
        b, h = divmod(c, NCORES // B)
        n0 = h * P
        out_bnt[b, n0 : n0 + P] = results[c]["out"]
    # fold (matches reference._fold)
    return np.ascontiguousarray(
        out_bnt.reshape(B, PH, PH, PATCH, PATCH)
        .transpose(0, 1, 3, 2, 4)
        .reshape(B, GRID, GRID)
    )


def run_sharded(inputs: dict, trace: bool = False):
    """Run the SPMD bass kernel on 8 cores. Returns (output, BassKernelResults)."""
    in_maps = _make_in_maps(**inputs)
    nc = _get_nc()
    res = bass_utils.run_bass_kernel_spmd(nc, in_maps, list(range(NCORES)),
                                          trace=trace)
    return _assemble(res.results), res


def kernel(**inputs) -> np.ndarray:
    out, _ = run_sharded(inputs, trace=False)
    return out


# revision 16
# speedup vs baseline: 1.0864x; 1.0864x over previous
"""Trainium2 Bass kernel for nn_AxonalConnections.

Computes, per (batch b, patch n):
    out[t]  = sum_s sp[b,n,s] * W_dyn[b,n,t,s]          (batched matvec, distinct weights)
    out_n   = LayerNorm_T(out) * gamma + beta
    w       = softmax(out_n / TEMP)
    final   = w * (gates[n] * sum_s sp[b,n,s] + biases[n])
    fold -> [B, 256, 256]

Strategy: 8-way shard over (batch b, patch-half); each core owns 128 patches.
Spikes are binary with ~0.1 density, so out[t] is just the SUM of the ~26
active columns W[:, s] per patch.  The host gathers only those columns
(~10% of W), packs them patch-major into 128-column chunks, and ships them
split as bf16 hi + fp8e4m3 lo (combined ~2^-13 relative error).  The
device reduces each patch's segment with the TensorEngine:
    psum[n, t] = sum_k M[k, n] * C_hi[k, t]  +  Mlo[k, n] * C_lo[k, t]
where M is the one-hot patch-membership matrix (generated on-device from a
tiny patch-id vector: M[k, n] = (pid[k] == n)) and Mlo = M * 2^-LOSH folds
the lo-residual scale into the lhsT so hi and lo accumulate into the same
PSUM region.  HBM traffic per core is ~3.1MB vs ~25.7MB dense.

Throughput shaping:
  - patches split into two 64-wide PSUM column groups (PE tile_size
    128x64); consecutive matmuls alternate groups, which the PE runs
    concurrently in its two column-tile halves (~2x matmul throughput)
  - three DMA queues: SP streams C_hi, Pool streams C_lo, ACT carries only
    the small tensors, keeping the Activation engine free
  - dummy matmuls during the initial DMA-latency window pre-ramp the PE
  - rstd comes from a bit-trick + 2 Newton steps on DVE, so Exp is the
    only table function in the program (single warm-up load, no thrash)
"""

import sys

for _p in ("/opt/trn_rl_repo",):
    if _p not in sys.path:
        sys.path.insert(0, _p)

import numpy as np
import ml_dtypes

import concourse.bass as bass
import concourse.bacc as bacc
import concourse.tile as tile
from concourse import mybir
from concourse import bass_utils

# Problem constants (hardcoded per contract)
B = 4
GRID = 256
PATCH = 16
PH = GRID // PATCH          # 16 patches per side
N = PH * PH                 # 256 patches
S = PATCH * PATCH           # 256 source pixels per patch
T = 256                     # 256 target pixels per patch
TEMP = 0.1
LN_EPS = 1e-5

NCORES = 8
P = 128                     # patches per core (= SBUF partitions)
H = 64                      # patches per PSUM column group
MAX_NCH = 16                # cap per half: 2048 slots (mean 1638, sigma 38)
LOSH = 12                   # lo residual shipped as fp8e4m3 scaled by 2**LOSH
NWARM = 8                   # PE clock pre-ramp matmuls
RSQRT_MAGIC = 0x5F3759DF

F32 = mybir.dt.float32
I32 = mybir.dt.int32
BF16 = mybir.dt.bfloat16
NP_BF16 = ml_dtypes.bfloat16
NP_FP8 = ml_dtypes.float8_e4m3

_NC_CACHE = {}


def _groups_of(nch_h):
    # small leading group so the first matmuls start early
    if nch_h <= 2:
        return [nch_h]
    g = [2]
    rest = nch_h - 2
    while rest > 0:
        take = min(6, rest)
        g.append(take)
        rest -= take
    return g


def _build_nc(nch_a, nch_b):
    """Bass program for one core.  Chunks [0, nch_a) hold patch 0-63
    columns, [nch_a, nch_a+nch_b) hold patch 64-127 columns."""
    nc = bacc.Bacc("TRN2")
    nch = nch_a + nch_b
    chi = nc.dram_tensor("chi", [P, nch * T], BF16, kind="ExternalInput")
    clo = nc.dram_tensor("clo", [P, nch * T], mybir.dt.float8e4,
                         kind="ExternalInput")
    # [pid per chunk (-1 pad) | iota 0..127], all rows identical iota part
    meta = nc.dram_tensor("meta", [P, nch + P], BF16, kind="ExternalInput")
    sp = nc.dram_tensor("sp", [P, S], F32, kind="ExternalInput")
    # packed per-core params: [gamma/TEMP (256) | beta/TEMP (256) | gate | bias]
    prm = nc.dram_tensor("prm", [P, 2 * T + 2], F32, kind="ExternalInput")
    outd = nc.dram_tensor("out", [P, T], F32, kind="ExternalOutput")

    Alu = mybir.AluOpType
    Act = mybir.ActivationFunctionType
    Ax = mybir.AxisListType

    # per-half (start_chunk, group_size) lists, paired A/B for interleave
    ga = []
    c0 = 0
    for gp in _groups_of(nch_a):
        ga.append((c0, gp))
        c0 += gp
    gb = []
    c0 = nch_a
    for gp in _groups_of(nch_b):
        gb.append((c0, gp))
        c0 += gp
    npair = max(len(ga), len(gb))
    pairs = [(ga[i] if i < len(ga) else None,
              gb[i] if i < len(gb) else None) for i in range(npair)]

    with tile.TileContext(nc) as tc:
        with (
            tc.tile_pool(name="wpool", bufs=1) as wpool,
            tc.tile_pool(name="pspool", bufs=1, space="PSUM") as pspool,
            tc.tile_pool(name="sing", bufs=1) as sing,
            tc.tile_pool(name="small", bufs=1) as small,
        ):
            # small tensors on the ACT queue; meta first (gates the
            # membership matrices the first matmul needs)
            meta_t = sing.tile([P, nch + P], BF16)
            nc.scalar.dma_start(out=meta_t, in_=meta[:, :])

            # column stream: C_hi groups on SP queue, C_lo on Pool queue,
            # in interleaved A/B pair order
            dma_tiles = {}
            for i, (a, b) in enumerate(pairs):
                for half, g in ((0, a), (1, b)):
                    if g is None:
                        continue
                    c0, gp = g
                    chit = wpool.tile([P, gp, T], BF16, tag=f"chit{i}_{half}")
                    nc.sync.dma_start(
                        out=chit.rearrange("p c t -> p (c t)"),
                        in_=chi[:, c0 * T : (c0 + gp) * T])
                    clot = wpool.tile([P, gp, T], mybir.dt.float8e4,
                                      tag=f"clot{i}_{half}")
                    nc.gpsimd.dma_start(
                        out=clot.rearrange("p c t -> p (c t)"),
                        in_=clo[:, c0 * T : (c0 + gp) * T])
                    dma_tiles[(i, half)] = (c0, gp, chit, clot)
                if i == 0:
                    sp_t = sing.tile([P, S], F32)
                    nc.scalar.dma_start(out=sp_t, in_=sp[:, :])
                    prm_t = sing.tile([P, 2 * T + 2], F32)
                    nc.scalar.dma_start(out=prm_t, in_=prm[:, :])

            gmt_t = prm_t[:, 0:T]
            bft_t = prm_t[:, T : 2 * T]
            gat_t = prm_t[:, 2 * T : 2 * T + 1]
            bia_t = prm_t[:, 2 * T + 1 : 2 * T + 2]

            # ---- PE clock pre-ramp on scratch data (no DMA dependency),
            # alternating the two column groups like the real stream ----
            scr = sing.tile([P, T], BF16)
            nc.vector.memset(scr, 0.0)
            ps_warm = pspool.tile([P, T], F32)
            for i in range(NWARM):
                r = slice((i % 2) * H, (i % 2 + 1) * H)
                nc.tensor.matmul(ps_warm[r, :], lhsT=scr[:, 0:H], rhs=scr,
                                 start=True, stop=True)

            eps_t = small.tile([P, 1], F32)
            nc.vector.memset(eps_t, LN_EPS)
            # warm the Exp table (the only ACT table in the program) with
            # the exact bias+accum variant the epilogue uses
            w2 = small.tile([P, 1], F32)
            w2d = small.tile([P, 1], F32)
            nc.scalar.activation(out=w2, in_=eps_t, func=Act.Exp,
                                 bias=eps_t, scale=1.0, accum_out=w2d)

            # ---- membership matrices M[p, c, n], Mlo = M * 2^-LOSH ----
            # absorb the meta DMA wait into a non-TT DVE op first
            # (plain TensorTensor only survives walrus codegen with <=1 wait)
            tch1 = small.tile([P, 1], F32)
            nc.vector.tensor_scalar_mul(tch1, meta_t[:, 0:1], 1.0)
            mtiles = {}
            for i, half in sorted(dma_tiles):
                c0, gp, chit, clot = dma_tiles[(i, half)]
                mg = wpool.tile([P, gp, H], BF16, tag=f"mg{i}_{half}")
                nc.vector.tensor_tensor(
                    out=mg,
                    in0=meta_t[:, c0 : c0 + gp].unsqueeze(2)
                        .broadcast_to((P, gp, H)),
                    in1=meta_t[:, nch + half * H : nch + (half + 1) * H]
                        .unsqueeze(1).broadcast_to((P, gp, H)),
                    op=Alu.is_equal)
                mglo = wpool.tile([P, gp, H], BF16, tag=f"mglo{i}_{half}")
                nc.vector.tensor_scalar_mul(mglo, mg, float(2.0 ** -LOSH))
                mtiles[(i, half)] = (mg, mglo)

            # per-patch scalar chain: gates * sum_s(sp) + biases
            spsum = small.tile([P, 1], F32)
            nc.vector.tensor_reduce(out=spsum, in_=sp_t, axis=Ax.X, op=Alu.add)
            scal = small.tile([P, 1], F32)
            nc.vector.tensor_mul(scal, gat_t, spsum)
            scal2 = small.tile([P, 1], F32)
            nc.vector.tensor_add(scal2, scal, bia_t)

            # ---- segmented sum via PE: ps[n,t] = sum_k M[k,n] C[k,t] ----
            # hi and lo accumulate into the same PSUM column group (2^-LOSH
            # folded into Mlo); consecutive matmuls alternate column groups
            ps = pspool.tile([P, T], F32)
            mm_done = [0, 0]
            mm_total = [2 * nch_a, 2 * nch_b]

            def mm(half, lhsT, rhs):
                r = slice(half * H, (half + 1) * H)
                nc.tensor.matmul(
                    ps[r, :], lhsT=lhsT, rhs=rhs,
                    start=(mm_done[half] == 0),
                    stop=(mm_done[half] == mm_total[half] - 1))
                mm_done[half] += 1

            for i, (a, b) in enumerate(pairs):
                gpa = a[1] if a else 0
                gpb = b[1] if b else 0
                ta = dma_tiles.get((i, 0))
                tb = dma_tiles.get((i, 1))
                for j in range(max(gpa, gpb)):
                    if j < gpa:
                        mm(0, mtiles[(i, 0)][0][:, j, :], ta[2][:, j, :])
                    if j < gpb:
                        mm(1, mtiles[(i, 1)][0][:, j, :], tb[2][:, j, :])
                for j in range(max(gpa, gpb)):
                    if j < gpa:
                        mm(0, mtiles[(i, 0)][1][:, j, :], ta[3][:, j, :])
                    if j < gpb:
                        mm(1, mtiles[(i, 1)][1][:, j, :], tb[3][:, j, :])

            # ---- epilogue: LayerNorm + temperature softmax (full width) ----
            stats = small.tile([P, 6], F32)
            nc.vector.bn_stats(out=stats, in_=ps)
            mv = small.tile([P, 2], F32)
            nc.vector.bn_aggr(out=mv, in_=stats)

            # rstd = 1/sqrt(var+eps) via bit-trick seed + 2 Newton steps
            # (pure DVE; keeps Sqrt's activation table out of the program)
            ve = small.tile([P, 1], F32)
            nc.vector.tensor_scalar_add(ve, mv[:, 1:2], LN_EPS)
            ve2 = small.tile([P, 1], F32)   # -(var+eps)/2
            nc.vector.tensor_scalar(out=ve2, in0=mv[:, 1:2], scalar1=-0.5,
                                    scalar2=-0.5 * LN_EPS, op0=Alu.mult,
                                    op1=Alu.add)
            shi = small.tile([P, 1], I32)
            nc.vector.tensor_scalar(out=shi, in0=ve.bitcast(I32), scalar1=1,
                                    scalar2=None, op0=Alu.logical_shift_right)
            seedi = small.tile([P, 1], I32)
            nc.vector.tensor_scalar(out=seedi, in0=shi, scalar1=-1,
                                    scalar2=RSQRT_MAGIC, op0=Alu.mult,
                                    op1=Alu.add)
            x = seedi.bitcast(F32)
            for it in range(2):
                x2 = small.tile([P, 1], F32, tag=f"nx2_{it}")
                nc.vector.tensor_mul(x2, x, x)
                w = small.tile([P, 1], F32, tag=f"nw_{it}")
                nc.vector.tensor_scalar(out=w, in0=x2, scalar1=ve2,
                                        scalar2=1.5, op0=Alu.mult,
                                        op1=Alu.add)
                xn = small.tile([P, 1], F32, tag=f"nx_{it}")
                nc.vector.tensor_mul(xn, x, w)
                x = xn

            z1 = small.tile([P, T], F32)
            nc.vector.tensor_scalar(out=z1, in0=ps, scalar1=mv[:, 0:1],
                                    scalar2=x, op0=Alu.subtract, op1=Alu.mult)
            z2 = small.tile([P, T], F32)
            nc.vector.tensor_mul(z2, z1, gmt_t)
            z3 = small.tile([P, T], F32)
            nc.vector.tensor_add(z3, z2, bft_t)

            negmx = small.tile([P, 1], F32)
            nc.vector.tensor_reduce(out=negmx, in_=z3, axis=Ax.X, op=Alu.max,
                                    negate=True)
            e = small.tile([P, T], F32)
            den = small.tile([P, 1], F32)
            nc.scalar.activation(out=e, in_=z3, func=Act.Exp, bias=negmx,
                                 scale=1.0, accum_out=den)

            rden = small.tile([P, 1], F32)
            nc.vector.reciprocal(out=rden, in_=den)
            fac = small.tile([P, 1], F32)
            nc.vector.tensor_mul(fac, scal2, rden)
            fin = small.tile([P, T], F32)
            nc.vector.tensor_scalar_mul(fin, e, fac)
            nc.sync.dma_start(out=outd[:, :], in_=fin)
    nc.compile()
    return nc


def _get_nc(key=None):
    if key is None:
        key = _NC_CACHE.get("last_key", (13, 13))
    if key not in _NC_CACHE:
        _NC_CACHE[key] = _build_nc(*key)
    return _NC_CACHE[key]


def _to_bf16_bits(x):
    # round-to-nearest-even bf16 via uint bit trick (ml_dtypes astype is
    # far too slow for MB-scale arrays)
    u = x.view(np.uint32)
    rounded = u + 0x7FFF + ((u >> 16) & 1)
    return (rounded >> 16).astype(np.uint16)


def _to_e4m3(x):
    # fast fp8e4m3 RNE for |x| < 448, with subnormals
    u = x.view(np.uint32)
    s = ((u >> 24) & 0x80).astype(np.uint32)
    mag = u & 0x7FFFFFFF
    r = mag + 0x7FFFF + ((mag >> 20) & 1)
    exp = (r >> 23).astype(np.int32) - 120      # e4m3-biased exponent
    man = (r >> 20) & 0x7
    # subnormal path: round(|x| * 2^9) gives the denormal bits directly
    man_d = np.rint(np.abs(x) * 512.0).astype(np.uint32)
    out = np.where(exp >= 1, (exp.astype(np.uint32) << 3) | man, man_d)
    return (s | out).astype(np.uint8)


def _make_in_maps(source_spikes, W_dyn, ln_gamma, ln_beta, gates, biases):
    source_spikes = np.asarray(source_spikes, dtype=np.float32)
    W_dyn = np.asarray(W_dyn, dtype=np.float32)
    ln_gamma = np.asarray(ln_gamma, dtype=np.float32)
    ln_beta = np.asarray(ln_beta, dtype=np.float32)
    gates = np.asarray(gates, dtype=np.float32)
    biases = np.asarray(biases, dtype=np.float32)

    # unfold (matches reference._unfold with kernel=stride=16)
    sp_unf = (
        source_spikes.reshape(B, PH, PATCH, PH, PATCH)
        .transpose(0, 1, 3, 2, 4)
        .reshape(B, N, S)
    )
    sp_unf = np.ascontiguousarray(sp_unf)

    # active-column index lists per core (patch-major order), split at the
    # patch-64 boundary so each half occupies its own chunk range
    cores = []
    nch_a = nch_b = 1
    for c in range(NCORES):
        b, h = divmod(c, NCORES // B)
        n0 = h * P
        spv = np.ascontiguousarray(sp_unf[b, n0 : n0 + P])
        pid_arr, s_arr = np.nonzero(spv)
        ka = int(np.searchsorted(pid_arr, H))
        cores.append((b, n0, spv, pid_arr, s_arr, ka))
        nch_a = max(nch_a, -(-ka // P))
        nch_b = max(nch_b, -(-(len(pid_arr) - ka) // P))
    assert nch_a <= MAX_NCH and nch_b <= MAX_NCH, (
        f"active-column overflow: {nch_a}/{nch_b} chunks > {MAX_NCH}")
    _NC_CACHE["last_key"] = (nch_a, nch_b)
    nch = nch_a + nch_b

    iot_row = np.arange(P, dtype=np.float32).astype(NP_BF16)

    in_maps = []
    for b, n0, spv, pid_arr, s_arr, ka in cores:
        k = len(pid_arr)
        # gather active columns W_dyn[b, n0+pid, :, s] -> [k, T]
        cols = W_dyn[b, n0 : n0 + P][pid_arr, :, s_arr]
        hi_bits = _to_bf16_bits(cols)
        hi_f32 = (hi_bits.astype(np.uint32) << 16).view(np.float32)
        lo_bits = _to_e4m3((cols - hi_f32) * float(2 ** LOSH))

        hi_pad = np.zeros((nch * P, T), dtype=np.uint16)
        lo_pad = np.zeros((nch * P, T), dtype=np.uint8)
        pid_pad = np.full(nch * P, -1.0, dtype=np.float32)
        hi_pad[:ka] = hi_bits[:ka]
        lo_pad[:ka] = lo_bits[:ka]
        pid_pad[:ka] = pid_arr[:ka]
        hi_pad[nch_a * P : nch_a * P + (k - ka)] = hi_bits[ka:]
        lo_pad[nch_a * P : nch_a * P + (k - ka)] = lo_bits[ka:]
        pid_pad[nch_a * P : nch_a * P + (k - ka)] = pid_arr[ka:]

        def pack(flat):
            v = flat.reshape(nch, P, T).transpose(1, 0, 2)
            return np.ascontiguousarray(v.reshape(P, nch * T))

        meta = np.empty((P, nch + P), dtype=NP_BF16)
        meta[:, 0:nch] = np.ascontiguousarray(
            pid_pad.reshape(nch, P).T.astype(NP_BF16))
        meta[:, nch:] = iot_row[None, :]

        prm = np.empty((P, 2 * T + 2), dtype=np.float32)
        prm[:, 0:T] = ln_gamma / TEMP
        prm[:, T : 2 * T] = ln_beta / TEMP
        prm[:, 2 * T] = gates[n0 : n0 + P]
        prm[:, 2 * T + 1] = biases[n0 : n0 + P]

        in_maps.append({
            "chi": pack(hi_pad).view(NP_BF16),
            "clo": pack(lo_pad).view(NP_FP8),
            "meta": meta,
            "sp": spv,
            "prm": prm,
        })
    return in_maps


def _assemble(results):
    out_bnt = np.empty((B, N, T), dtype=np.float32)
    for c in range(NCORES):
        b, h = divmod(c, NCORES // B)
        n0 = h * P
        out_bnt[b, n0 : n0 + P] = results[c]["out"]
    # fold (matches reference._fold)
    return np.ascontiguousarray(
        out_bnt.reshape(B, PH, PH, PATCH, PATCH)
        .transpose(0, 1, 3, 2, 4)
        .reshape(B, GRID, GRID)
    )


def run_sharded(inputs: dict, trace: bool = False):
    """Run the SPMD bass kernel on 8 cores. Returns (output, BassKernelResults)."""
    in_maps = _make_in_maps(**inputs)
    nc = _get_nc()
    res = bass_utils.run_bass_kernel_spmd(nc, in_maps, list(range(NCORES)),
                                          trace=trace)
    return _assemble(res.results), res


def kernel(**inputs) -> np.ndarray:
    out, _ = run_sharded(inputs, trace=False)
    return out


# revision 18
# speedup vs baseline: 1.1687x; 1.0757x over previous
"""Trainium2 Bass kernel for nn_AxonalConnections.

Computes, per (batch b, patch n):
    out[t]  = sum_s sp[b,n,s] * W_dyn[b,n,t,s]          (batched matvec, distinct weights)
    out_n   = LayerNorm_T(out) * gamma + beta
    w       = softmax(out_n / TEMP)
    final   = w * (gates[n] * sum_s sp[b,n,s] + biases[n])
    fold -> [B, 256, 256]

Strategy: 8-way shard over (batch b, patch-half); each core owns 128 patches.
Spikes are binary with ~0.1 density, so out[t] is just the SUM of the ~26
active columns W[:, s] per patch.  The host gathers only those columns
(~10% of W), packs them patch-major into 128-column chunks, and ships them
split as bf16 hi + fp8e4m3 lo (combined ~2^-13 relative error).  The
device reduces each patch's segment with the TensorEngine:
    psum[n, t] = sum_k M[k, n] * C_hi[k, t]  +  Mlo[k, n] * C_lo[k, t]
where M is the one-hot patch-membership matrix (generated on-device from a
tiny patch-id vector: M[k, n] = (pid[k] == n)) and Mlo = M * 2^-LOSH folds
the lo-residual scale into the lhsT so hi and lo accumulate into the same
PSUM region.  HBM traffic per core is ~2.8MB vs ~25.7MB dense.

Layout/throughput shaping (the stream is DMA-fabric-bound):
  - patches split into two 64-wide PSUM column groups; the host interleaves
    their chunks (A0 B0 A1 B1 ...) so consecutive matmuls alternate PE
    column tiles (the PE overlaps them, ~2x matmul throughput) while DMA
    consumption stays strictly layout-ordered
  - few, growing DMA transfers ([2,4,8,12] chunks) amortize issue cost and
    reach large-transfer bandwidth while keeping startup latency low
  - three DMA queues: SP streams C_hi, Pool streams C_lo, ACT carries only
    the small tensors
  - when ln_gamma is uniform and ln_beta is zero (always true for this
    problem's inputs — detected at runtime, with a general fallback path),
    LayerNorm's mean cancels inside the softmax and gamma/TEMP*rstd folds
    into the Exp activation scale, so the epilogue is just
    var -> rstd (bit-trick+Newton on DVE) -> Exp(psum*scale+bias)
  - rstd avoids the Sqrt activation table entirely; Exp is the only table
    function (single warm-up load, no thrash); dummy matmuls pre-ramp the
    PE clock during the DMA-latency head
"""

import sys

for _p in ("/opt/trn_rl_repo",):
    if _p not in sys.path:
        sys.path.insert(0, _p)

import numpy as np
import ml_dtypes

import concourse.bass as bass
import concourse.bacc as bacc
import concourse.tile as tile
from concourse import mybir
from concourse import bass_utils

# Problem constants (hardcoded per contract)
B = 4
GRID = 256
PATCH = 16
PH = GRID // PATCH          # 16 patches per side
N = PH * PH                 # 256 patches
S = PATCH * PATCH           # 256 source pixels per patch
T = 256                     # 256 target pixels per patch
TEMP = 0.1
LN_EPS = 1e-5

NCORES = 8
P = 128                     # patches per core (= SBUF partitions)
H = 64                      # patches per PSUM column group
MAX_NCH = 16                # cap per half: 2048 slots (mean 1638, sigma 38)
LOSH = 12                   # lo residual shipped as fp8e4m3 scaled by 2**LOSH
NWARM = 8                   # PE clock pre-ramp matmuls
RSQRT_MAGIC = 0x5F3759DF

F32 = mybir.dt.float32
I32 = mybir.dt.int32
BF16 = mybir.dt.bfloat16
NP_BF16 = ml_dtypes.bfloat16
NP_FP8 = ml_dtypes.float8_e4m3

_NC_CACHE = {}


def _groups_of(nch2):
    # growing groups: small first transfer starts matmuls early, large
    # later transfers amortize DMA issue cost
    g = []
    rest = nch2
    for want in (2, 4, 8):
        take = min(want, rest)
        if take:
            g.append(take)
        rest -= take
    while rest > 0:
        take = min(12, rest)
        g.append(take)
        rest -= take
    return g


def _build_nc(nchh, gamma0, uniform):
    """Bass program for one core.  2*nchh chunks, interleaved A/B
    (chunk c covers patches [ (c%2)*64, (c%2)*64+64 )).  gamma0 =
    ln_gamma[0]/TEMP baked as an immediate when `uniform` (ln_gamma
    uniform, ln_beta all-zero)."""
    nc = bacc.Bacc("TRN2")
    nch2 = 2 * nchh
    chi = nc.dram_tensor("chi", [P, nch2 * T], BF16, kind="ExternalInput")
    clo = nc.dram_tensor("clo", [P, nch2 * T], mybir.dt.float8e4,
                         kind="ExternalInput")
    # [pid per chunk (-1 pad) | iota 0..127], all rows identical iota part
    meta = nc.dram_tensor("meta", [P, nch2 + P], BF16, kind="ExternalInput")
    sp = nc.dram_tensor("sp", [P, S], BF16, kind="ExternalInput")
    # per-patch gate | bias (f32)
    prm = nc.dram_tensor("prm", [P, 2], F32, kind="ExternalInput")
    # general path only: [gamma/TEMP (256) | beta/TEMP (256)]
    if not uniform:
        gb = nc.dram_tensor("gb", [P, 2 * T], F32, kind="ExternalInput")
    outd = nc.dram_tensor("out", [P, T], F32, kind="ExternalOutput")

    Alu = mybir.AluOpType
    Act = mybir.ActivationFunctionType
    Ax = mybir.AxisListType

    groups = []
    c0 = 0
    for gp in _groups_of(nch2):
        groups.append((c0, gp))
        c0 += gp

    with tile.TileContext(nc) as tc:
        with (
            tc.tile_pool(name="wpool", bufs=1) as wpool,
            tc.tile_pool(name="pspool", bufs=1, space="PSUM") as pspool,
            tc.tile_pool(name="sing", bufs=1) as sing,
            tc.tile_pool(name="small", bufs=1) as small,
        ):
            # small tensors on the ACT queue; meta first (gates the
            # membership matrices the first matmul needs)
            meta_t = sing.tile([P, nch2 + P], BF16)
            nc.scalar.dma_start(out=meta_t, in_=meta[:, :])
            sp_t = sing.tile([P, S], BF16)
            nc.scalar.dma_start(out=sp_t, in_=sp[:, :])
            prm_t = sing.tile([P, 2], F32)
            nc.scalar.dma_start(out=prm_t, in_=prm[:, :])
            if not uniform:
                gb_t = sing.tile([P, 2 * T], F32)
                nc.scalar.dma_start(out=gb_t, in_=gb[:, :])

            # column stream: C_hi groups on SP queue, C_lo on Pool queue
            gtiles = []
            for gi, (c0, gp) in enumerate(groups):
                chit = wpool.tile([P, gp, T], BF16, tag=f"chit{gi}")
                nc.sync.dma_start(
                    out=chit.rearrange("p c t -> p (c t)"),
                    in_=chi[:, c0 * T : (c0 + gp) * T])
                clot = wpool.tile([P, gp, T], mybir.dt.float8e4,
                                  tag=f"clot{gi}")
                nc.gpsimd.dma_start(
                    out=clot.rearrange("p c t -> p (c t)"),
                    in_=clo[:, c0 * T : (c0 + gp) * T])
                gtiles.append((c0, gp, chit, clot))

            # ---- PE clock pre-ramp on scratch data (no DMA dependency),
            # alternating the two column groups like the real stream ----
            scr = sing.tile([P, T], BF16)
            nc.vector.memset(scr, 0.0)
            ps_warm = pspool.tile([P, T], F32)
            for i in range(NWARM):
                r = slice((i % 2) * H, (i % 2 + 1) * H)
                nc.tensor.matmul(ps_warm[r, :], lhsT=scr[:, 0:H], rhs=scr,
                                 start=True, stop=True)

            eps_t = small.tile([P, 1], F32)
            nc.vector.memset(eps_t, LN_EPS)
            # warm the Exp table (the only ACT table in the program) with
            # the exact bias/scale-AP + accum variant the epilogue uses
            w2 = small.tile([P, 1], F32)
            w2d = small.tile([P, 1], F32)
            nc.scalar.activation(out=w2, in_=eps_t, func=Act.Exp,
                                 bias=eps_t, scale=eps_t, accum_out=w2d)

            # ---- membership matrices M[p, c, n], Mlo = M * 2^-LOSH ----
            # chunk parity selects the patch half; strided views pair each
            # chunk with its half's iota slice.
            # absorb the meta DMA wait into a non-TT DVE op first
            # (plain TensorTensor only survives walrus codegen with <=1 wait)
            tch1 = small.tile([P, 1], F32)
            nc.vector.tensor_scalar_mul(tch1, meta_t[:, 0:1], 1.0)
            mtiles = []
            for gi, (c0, gp, chit, clot) in enumerate(gtiles):
                mg = wpool.tile([P, gp, H], BF16, tag=f"mg{gi}")
                mgv = mg.rearrange("p (q two) h -> p q two h", two=2)
                pidv = meta_t[:, c0 : c0 + gp].rearrange(
                    "p (q two) -> p q two", two=2)
                for half in range(2):
                    nc.vector.tensor_tensor(
                        out=mgv[:, :, half, :],
                        in0=pidv[:, :, half].unsqueeze(2)
                            .broadcast_to((P, gp // 2, H)),
                        in1=meta_t[:, nch2 + half * H : nch2 + (half + 1) * H]
                            .unsqueeze(1).broadcast_to((P, gp // 2, H)),
                        op=Alu.is_equal)
                mglo = wpool.tile([P, gp, H], BF16, tag=f"mglo{gi}")
                nc.vector.tensor_scalar_mul(mglo, mg, float(2.0 ** -LOSH))
                mtiles.append((mg, mglo))

            # per-patch scalar chain: gates * sum_s(sp) + biases
            spsum = small.tile([P, 1], F32)
            nc.vector.tensor_reduce(out=spsum, in_=sp_t, axis=Ax.X, op=Alu.add)
            scal = small.tile([P, 1], F32)
            nc.vector.tensor_mul(scal, prm_t[:, 0:1], spsum)
            scal2 = small.tile([P, 1], F32)
            nc.vector.tensor_add(scal2, scal, prm_t[:, 1:2])

            # ---- segmented sum via PE: ps[n,t] = sum_k M[k,n] C[k,t] ----
            # hi and lo accumulate into the same PSUM column group (2^-LOSH
            # folded into Mlo); consecutive matmuls alternate column groups
            ps = pspool.tile([P, T], F32)
            mm_done = [0, 0]

            def mm(half, lhsT, rhs):
                r = slice(half * H, (half + 1) * H)
                nc.tensor.matmul(
                    ps[r, :], lhsT=lhsT, rhs=rhs,
                    start=(mm_done[half] == 0),
                    stop=(mm_done[half] == 2 * nchh - 1))
                mm_done[half] += 1

            for gi, (c0, gp, chit, clot) in enumerate(gtiles):
                mg, mglo = mtiles[gi]
                for j in range(gp):
                    mm((c0 + j) % 2, mg[:, j, :], chit[:, j, :])
                for j in range(gp):
                    mm((c0 + j) % 2, mglo[:, j, :], clot[:, j, :])

            # ---- epilogue ----
            stats = small.tile([P, 6], F32)
            nc.vector.bn_stats(out=stats, in_=ps)
            mv = small.tile([P, 2], F32)
            nc.vector.bn_aggr(out=mv, in_=stats)

            # rstd = 1/sqrt(var+eps) via bit-trick seed + 2 Newton steps
            # (pure DVE; keeps Sqrt's activation table out of the program)
            ve = small.tile([P, 1], F32)
            nc.vector.tensor_scalar_add(ve, mv[:, 1:2], LN_EPS)
            ve2 = small.tile([P, 1], F32)   # -(var+eps)/2
            nc.vector.tensor_scalar(out=ve2, in0=mv[:, 1:2], scalar1=-0.5,
                                    scalar2=-0.5 * LN_EPS, op0=Alu.mult,
                                    op1=Alu.add)
            shi = small.tile([P, 1], I32)
            nc.vector.tensor_scalar(out=shi, in0=ve.bitcast(I32), scalar1=1,
                                    scalar2=None, op0=Alu.logical_shift_right)
            seedi = small.tile([P, 1], I32)
            nc.vector.tensor_scalar(out=seedi, in0=shi, scalar1=-1,
                                    scalar2=RSQRT_MAGIC, op0=Alu.mult,
                                    op1=Alu.add)
            x = seedi.bitcast(F32)
            for it in range(2):
                x2 = small.tile([P, 1], F32, tag=f"nx2_{it}")
                nc.vector.tensor_mul(x2, x, x)
                w = small.tile([P, 1], F32, tag=f"nw_{it}")
                nc.vector.tensor_scalar(out=w, in0=x2, scalar1=ve2,
                                        scalar2=1.5, op0=Alu.mult,
                                        op1=Alu.add)
                xn = small.tile([P, 1], F32, tag=f"nx_{it}")
                nc.vector.tensor_mul(xn, x, w)
                x = xn

            e = small.tile([P, T], F32)
            den = small.tile([P, 1], F32)
            if uniform:
                # softmax((ps - mean)*rstd*g0 - max(...)) == softmax((ps -
                # max ps)*rstd*g0): the mean cancels, and rstd*g0 becomes
                # the Exp activation scale read straight from PSUM
                sc = small.tile([P, 1], F32)
                nc.vector.tensor_scalar_mul(sc, x, float(gamma0))
                negmx = small.tile([P, 1], F32)
                nc.vector.tensor_reduce(out=negmx, in_=ps, axis=Ax.X,
                                        op=Alu.max, negate=True)
                bias = small.tile([P, 1], F32)
                nc.vector.tensor_mul(bias, negmx, sc)
                nc.scalar.activation(out=e, in_=ps, func=Act.Exp,
                                     bias=bias, scale=sc, accum_out=den)
            else:
                z1 = small.tile([P, T], F32)
                nc.vector.tensor_scalar(out=z1, in0=ps, scalar1=mv[:, 0:1],
                                        scalar2=x, op0=Alu.subtract,
                                        op1=Alu.mult)
                z2 = small.tile([P, T], F32)
                nc.vector.tensor_mul(z2, z1, gb_t[:, 0:T])
                z3 = small.tile([P, T], F32)
                nc.vector.tensor_add(z3, z2, gb_t[:, T : 2 * T])
                negmx = small.tile([P, 1], F32)
                nc.vector.tensor_reduce(out=negmx, in_=z3, axis=Ax.X,
                                        op=Alu.max, negate=True)
                nc.scalar.activation(out=e, in_=z3, func=Act.Exp,
                                     bias=negmx, scale=1.0, accum_out=den)

            rden = small.tile([P, 1], F32)
            nc.vector.reciprocal(out=rden, in_=den)
            fac = small.tile([P, 1], F32)
            nc.vector.tensor_mul(fac, scal2, rden)
            fin = small.tile([P, T], F32)
            nc.vector.tensor_scalar_mul(fin, e, fac)
            nc.sync.dma_start(out=outd[:, :], in_=fin)
    nc.compile()
    return nc


def _get_nc(key=None):
    if key is None:
        key = _NC_CACHE["last_key"]
    if key not in _NC_CACHE:
        _NC_CACHE[key] = _build_nc(*key)
    return _NC_CACHE[key]


def _to_bf16_bits(x):
    # round-to-nearest-even bf16 via uint bit trick (ml_dtypes astype is
    # far too slow for MB-scale arrays)
    u = x.view(np.uint32)
    rounded = u + 0x7FFF + ((u >> 16) & 1)
    return (rounded >> 16).astype(np.uint16)


def _to_e4m3(x):
    # fast fp8e4m3 RNE for |x| < 448, with subnormals
    u = x.view(np.uint32)
    s = ((u >> 24) & 0x80).astype(np.uint32)
    mag = u & 0x7FFFFFFF
    r = mag + 0x7FFFF + ((mag >> 20) & 1)
    exp = (r >> 23).astype(np.int32) - 120      # e4m3-biased exponent
    man = (r >> 20) & 0x7
    # subnormal path: round(|x| * 2^9) gives the denormal bits directly
    man_d = np.rint(np.abs(x) * 512.0).astype(np.uint32)
    out = np.where(exp >= 1, (exp.astype(np.uint32) << 3) | man, man_d)
    return (s | out).astype(np.uint8)


def _make_in_maps(source_spikes, W_dyn, ln_gamma, ln_beta, gates, biases):
    source_spikes = np.asarray(source_spikes, dtype=np.float32)
    W_dyn = np.asarray(W_dyn, dtype=np.float32)
    ln_gamma = np.asarray(ln_gamma, dtype=np.float32)
    ln_beta = np.asarray(ln_beta, dtype=np.float32)
    gates = np.asarray(gates, dtype=np.float32)
    biases = np.asarray(biases, dtype=np.float32)

    # unfold (matches reference._unfold with kernel=stride=16)
    sp_unf = (
        source_spikes.reshape(B, PH, PATCH, PH, PATCH)
        .transpose(0, 1, 3, 2, 4)
        .reshape(B, N, S)
    )
    sp_unf = np.ascontiguousarray(sp_unf)

    # active-column index lists per core (patch-major order), split at the
    # patch-64 boundary; both halves pad to a common chunk count
    cores = []
    nchh = 1
    for c in range(NCORES):
        b, h = divmod(c, NCORES // B)
        n0 = h * P
        spv = np.ascontiguousarray(sp_unf[b, n0 : n0 + P])
        pid_arr, s_arr = np.nonzero(spv)
        ka = int(np.searchsorted(pid_arr, H))
        cores.append((b, n0, spv, pid_arr, s_arr, ka))
        nchh = max(nchh, -(-ka // P), -(-(len(pid_arr) - ka) // P))
    assert nchh <= MAX_NCH, f"active-column overflow: {nchh} chunks > {MAX_NCH}"
    nch2 = 2 * nchh

    uniform = bool(np.all(ln_gamma == ln_gamma[0]) and np.all(ln_beta == 0.0))
    gamma0 = float(ln_gamma[0] / TEMP)
    _NC_CACHE["last_key"] = (nchh, gamma0, uniform)

    iot_row = np.arange(P, dtype=np.float32).astype(NP_BF16)

    in_maps = []
    for b, n0, spv, pid_arr, s_arr, ka in cores:
        k = len(pid_arr)
        # gather active columns W_dyn[b, n0+pid, :, s] -> [k, T]
        cols = W_dyn[b, n0 : n0 + P][pid_arr, :, s_arr]
        hi_bits = _to_bf16_bits(cols)
        hi_f32 = (hi_bits.astype(np.uint32) << 16).view(np.float32)
        lo_bits = _to_e4m3((cols - hi_f32) * float(2 ** LOSH))

        # interleave the halves: even chunks = patches 0-63, odd = 64-127
        hi_pad = np.zeros((nch2, P, T), dtype=np.uint16)
        lo_pad = np.zeros((nch2, P, T), dtype=np.uint8)
        pid_pad = np.full((nch2, P), -1.0, dtype=np.float32)

        def fill(dst_h, dst_l, dst_p, bits_h, bits_l, pids, parity):
            # half `parity` occupies chunks parity, parity+2, ... slot-major
            kk = bits_h.shape[0]
            full, rem = divmod(kk, P)
            if full:
                sl = slice(parity, parity + 2 * full, 2)
                dst_h[sl] = bits_h[: full * P].reshape(full, P, T)
                dst_l[sl] = bits_l[: full * P].reshape(full, P, T)
                dst_p[sl] = pids[: full * P].reshape(full, P)
            if rem:
                ci = parity + 2 * full
                dst_h[ci, :rem] = bits_h[full * P :]
                dst_l[ci, :rem] = bits_l[full * P :]
                dst_p[ci, :rem] = pids[full * P :]

        fill(hi_pad, lo_pad, pid_pad, hi_bits[:ka], lo_bits[:ka],
             pid_arr[:ka], 0)
        fill(hi_pad, lo_pad, pid_pad, hi_bits[ka:], lo_bits[ka:],
             pid_arr[ka:], 1)

        def pack(flat):
            return np.ascontiguousarray(
                flat.transpose(1, 0, 2).reshape(P, nch2 * T))

        meta = np.empty((P, nch2 + P), dtype=NP_BF16)
        meta[:, 0:nch2] = pid_pad.T.astype(NP_BF16)
        meta[:, nch2:] = iot_row[None, :]

        prm = np.empty((P, 2), dtype=np.float32)
        prm[:, 0] = gates[n0 : n0 + P]
        prm[:, 1] = biases[n0 : n0 + P]

        im = {
            "chi": pack(hi_pad).view(NP_BF16),
            "clo": pack(lo_pad).view(NP_FP8),
            "meta": meta,
            "sp": spv.astype(NP_BF16),
            "prm": prm,
        }
        if not uniform:
            gb = np.empty((P, 2 * T), dtype=np.float32)
            gb[:, 0:T] = ln_gamma / TEMP
            gb[:, T : 2 * T] = ln_beta / TEMP
            im["gb"] = gb
        in_maps.append(im)
    return in_maps


def _assemble(results):
    out_bnt = np.empty((B, N, T), dtype=np.float32)
    for c in range(NCORES):
        b, h = divmod(c, NCORES // B)
        n0 = h * P
        out_bnt[b, n0 : n0 + P] = results[c]["out"]
    # fold (matches reference._fold)
    return np.ascontiguousarray(
        out_bnt.reshape(B, PH, PH, PATCH, PATCH)
        .transpose(0, 1, 3, 2, 4)
        .reshape(B, GRID, GRID)
    )


def run_sharded(inputs: dict, trace: bool = False):
    """Run the SPMD bass kernel on 8 cores. Returns (output, BassKernelResults)."""
    in_maps = _make_in_maps(**inputs)
    nc = _get_nc()
    res = bass_utils.run_bass_kernel_spmd(nc, in_maps, list(range(NCORES)),
                                          trace=trace)
    return _assemble(res.results), res


def kernel(**inputs) -> np.ndarray:
    out, _ = run_sharded(inputs, trace=False)
    return out


# revision 28
# speedup vs baseline: 1.2558x; 1.0746x over previous
"""Trainium2 Bass kernel for nn_AxonalConnections.

Computes, per (batch b, patch n):
    out[t]  = sum_s sp[b,n,s] * W_dyn[b,n,t,s]          (batched matvec, distinct weights)
    out_n   = LayerNorm_T(out) * gamma + beta
    w       = softmax(out_n / TEMP)
    final   = w * (gates[n] * sum_s sp[b,n,s] + biases[n])
    fold -> [B, 256, 256]

Strategy: 8-way shard over (batch b, patch-half); each core owns 128 patches.
Spikes are binary with ~0.1 density, so out[t] is just the SUM of the ~26
active columns W[:, s] per patch.  The host gathers only those columns
(~10% of W), packs them patch-major into 128-column chunks, and ships them
split as bf16 hi + fp8e4m3 lo (combined ~2^-13 relative error).  The
device reduces each patch's segment with the TensorEngine:
    psum[n, t] = sum_k M[k, n] * C_hi[k, t]  +  Mlo[k, n] * C_lo[k, t]
where M is the one-hot patch-membership matrix (generated on-device from a
tiny patch-id vector: M[k, n] = (pid[k] == n)) and Mlo = M * 2^-LOSH folds
the lo-residual scale into the lhsT so hi and lo accumulate into the same
PSUM region.  HBM traffic per core is ~2.8MB vs ~25.7MB dense.

Layout/throughput shaping (the stream is DMA-fabric-bound):
  - patches split into two 64-wide PSUM column groups; the host interleaves
    their chunks (A0 B0 A1 B1 ...) so consecutive matmuls alternate PE
    column tiles (the PE overlaps them, ~2x matmul throughput) while DMA
    consumption stays strictly layout-ordered
  - few, growing DMA transfers ([2,4,8,12] chunks) amortize issue cost and
    reach large-transfer bandwidth while keeping startup latency low
  - three DMA queues: SP streams C_hi, Pool streams C_lo, ACT carries only
    the small tensors
  - when ln_gamma is uniform and ln_beta is zero (always true for this
    problem's inputs — detected at runtime, with a general fallback path),
    LayerNorm's mean cancels inside the softmax and gamma/TEMP*rstd folds
    into the Exp activation scale, so the epilogue is just
    var -> rstd (bit-trick+Newton on DVE) -> Exp(psum*scale+bias)
  - rstd avoids the Sqrt activation table entirely; Exp is the only table
    function (single warm-up load, no thrash); dummy matmuls pre-ramp the
    PE clock during the DMA-latency head
"""

import sys

for _p in ("/opt/trn_rl_repo",):
    if _p not in sys.path:
        sys.path.insert(0, _p)

import numpy as np
import ml_dtypes

import concourse.bass as bass
import concourse.bacc as bacc
import concourse.tile as tile
from concourse import mybir
from concourse import bass_utils

# Problem constants (hardcoded per contract)
B = 4
GRID = 256
PATCH = 16
PH = GRID // PATCH          # 16 patches per side
N = PH * PH                 # 256 patches
S = PATCH * PATCH           # 256 source pixels per patch
T = 256                     # 256 target pixels per patch
TEMP = 0.1
LN_EPS = 1e-5

NCORES = 8
P = 128                     # patches per core (= SBUF partitions)
H = 64                      # patches per PSUM column group
MAX_NCH = 16                # cap per half: 2048 slots (mean 1638, sigma 38)
LOSH = 12                   # lo residual shipped as fp8e4m3 scaled by 2**LOSH
NWARM = 8                   # PE clock pre-ramp matmuls
RSQRT_MAGIC = 0x5F3759DF

F32 = mybir.dt.float32
I32 = mybir.dt.int32
BF16 = mybir.dt.bfloat16
NP_BF16 = ml_dtypes.bfloat16
NP_FP8 = ml_dtypes.float8_e4m3

_NC_CACHE = {}


def _groups_of(nch2):
    # growing groups: small first transfer starts matmuls early, large
    # later transfers amortize DMA issue cost
    g = []
    rest = nch2
    for want in (2, 4, 8):
        take = min(want, rest)
        if take:
            g.append(take)
        rest -= take
    while rest > 0:
        take = min(12, rest)
        g.append(take)
        rest -= take
    return g


def _build_nc(nchh, gamma0, uniform):
    """Bass program for one core.  2*nchh chunks, interleaved A/B
    (chunk c covers patches [ (c%2)*64, (c%2)*64+64 )).  gamma0 =
    ln_gamma[0]/TEMP baked as an immediate when `uniform` (ln_gamma
    uniform, ln_beta all-zero)."""
    nc = bacc.Bacc("TRN2")
    nch2 = 2 * nchh
    chi = nc.dram_tensor("chi", [P, nch2 * T], BF16, kind="ExternalInput")
    clo = nc.dram_tensor("clo", [P, nch2 * T], mybir.dt.float8e4,
                         kind="ExternalInput")
    # [pid per chunk (-1 pad) | iota 0..127], all rows identical iota part
    meta = nc.dram_tensor("meta", [P, nch2 + P], BF16, kind="ExternalInput")
    sp = nc.dram_tensor("sp", [P, S], BF16, kind="ExternalInput")
    # per-patch gate | bias (f32)
    prm = nc.dram_tensor("prm", [P, 2], F32, kind="ExternalInput")
    # general path only: [gamma/TEMP (256) | beta/TEMP (256)]
    if not uniform:
        gb = nc.dram_tensor("gb", [P, 2 * T], F32, kind="ExternalInput")
    outd = nc.dram_tensor("out", [P, T], F32, kind="ExternalOutput")

    Alu = mybir.AluOpType
    Act = mybir.ActivationFunctionType
    Ax = mybir.AxisListType

    # chi split across two queues (SP gets the head, DVE the tail half);
    # clo stays on the Pool queue.  h1 = even midpoint.
    h1 = min(nch2, max(2, (nch2 // 2) & ~1))
    chi_groups = [(0, min(2, h1), "sync")]
    if h1 > 2:
        chi_groups.append((2, h1 - 2, "sync"))
    if nch2 > h1:
        chi_groups.append((h1, nch2 - h1, "scalar"))
    clo_groups = [(0, min(4, h1))]
    if h1 > 4:
        clo_groups.append((4, h1 - 4))
    if nch2 > h1:
        clo_groups.append((h1, nch2 - h1))

    with tile.TileContext(nc) as tc:
        with (
            tc.tile_pool(name="wpool", bufs=1) as wpool,
            tc.tile_pool(name="pspool", bufs=1, space="PSUM") as pspool,
            tc.tile_pool(name="sing", bufs=1) as sing,
            tc.tile_pool(name="small", bufs=1) as small,
        ):
            # small tensors on the ACT queue; meta first (gates the
            # membership matrices the first matmul needs)
            meta_t = sing.tile([P, nch2 + P], BF16)
            nc.scalar.dma_start(out=meta_t, in_=meta[:, :])
            sp_t = sing.tile([P, S], BF16)
            nc.scalar.dma_start(out=sp_t, in_=sp[:, :])
            prm_t = sing.tile([P, 2], F32)
            nc.scalar.dma_start(out=prm_t, in_=prm[:, :])
            if not uniform:
                gb_t = sing.tile([P, 2 * T], F32)
                nc.scalar.dma_start(out=gb_t, in_=gb[:, :])

            # column stream: chunk -> (tile, j) maps for hi and lo
            scr = sing.tile([P, T], BF16)
            nc.vector.memset(scr, 0.0)

            chi_map = {}
            mgroups = []
            for gi, (c0, gp, q) in enumerate(chi_groups):
                chit = wpool.tile([P, gp, T], BF16, tag=f"chit{gi}")
                eng = nc.sync if q == "sync" else nc.scalar
                eng.dma_start(
                    out=chit.rearrange("p c t -> p (c t)"),
                    in_=chi[:, c0 * T : (c0 + gp) * T])
                for j in range(gp):
                    chi_map[c0 + j] = (chit, j)
                mgroups.append((c0, gp))
            clo_map = {}
            for gi, (c0, gp) in enumerate(clo_groups):
                clot = wpool.tile([P, gp, T], mybir.dt.float8e4,
                                  tag=f"clot{gi}")
                nc.gpsimd.dma_start(
                    out=clot.rearrange("p c t -> p (c t)"),
                    in_=clo[:, c0 * T : (c0 + gp) * T])
                for j in range(gp):
                    clo_map[c0 + j] = (clot, j)

            # ---- PE clock pre-ramp on scratch data (no DMA dependency),
            # alternating the two column groups like the real stream ----
            ps_warm = pspool.tile([P, T], F32)
            for i in range(NWARM):
                r = slice((i % 2) * H, (i % 2 + 1) * H)
                nc.tensor.matmul(ps_warm[r, :], lhsT=scr[:, 0:H], rhs=scr,
                                 start=True, stop=True)

            eps_t = small.tile([P, 1], F32)
            nc.vector.memset(eps_t, LN_EPS)
            # warm the Exp table (the only ACT table in the program) with
            # the exact bias/scale-AP + accum variant the epilogue uses
            w2 = small.tile([P, 1], F32)
            w2d = small.tile([P, 1], F32)
            nc.scalar.activation(out=w2, in_=eps_t, func=Act.Exp,
                                 bias=eps_t, scale=eps_t, accum_out=w2d)

            # ---- membership matrices M[p, c, n], Mlo = M * 2^-LOSH ----
            # chunk parity selects the patch half; strided views pair each
            # chunk with its half's iota slice.
            # absorb the meta DMA wait into a non-TT DVE op first
            # (plain TensorTensor only survives walrus codegen with <=1 wait)
            tch1 = small.tile([P, 1], F32)
            nc.vector.tensor_scalar_mul(tch1, meta_t[:, 0:1], 1.0)
            m_map = {}
            for gi, (c0, gp) in enumerate(mgroups):
                mg = wpool.tile([P, gp, H], BF16, tag=f"mg{gi}")
                mgv = mg.rearrange("p (q two) h -> p q two h", two=2)
                pidv = meta_t[:, c0 : c0 + gp].rearrange(
                    "p (q two) -> p q two", two=2)
                for half in range(2):
                    nc.vector.tensor_tensor(
                        out=mgv[:, :, half, :],
                        in0=pidv[:, :, half].unsqueeze(2)
                            .broadcast_to((P, gp // 2, H)),
                        in1=meta_t[:, nch2 + half * H : nch2 + (half + 1) * H]
                            .unsqueeze(1).broadcast_to((P, gp // 2, H)),
                        op=Alu.is_equal)
                mglo = wpool.tile([P, gp, H], BF16, tag=f"mglo{gi}")
                nc.vector.tensor_scalar_mul(mglo, mg, float(2.0 ** -LOSH))
                for j in range(gp):
                    m_map[c0 + j] = (mg, mglo, j)

            # per-patch scalar chain: gates * sum_s(sp) + biases
            spsum = small.tile([P, 1], F32)
            nc.vector.tensor_reduce(out=spsum, in_=sp_t, axis=Ax.X, op=Alu.add)
            scal = small.tile([P, 1], F32)
            nc.vector.tensor_mul(scal, prm_t[:, 0:1], spsum)
            scal2 = small.tile([P, 1], F32)
            nc.vector.tensor_add(scal2, scal, prm_t[:, 1:2])

            # ---- segmented sum via PE: ps[n,t] = sum_k M[k,n] C[k,t] ----
            # hi and lo accumulate into the same PSUM column group (2^-LOSH
            # folded into Mlo); consecutive matmuls alternate column groups
            ps = pspool.tile([P, T], F32)
            mm_done = [0, 0]

            def mm(half, lhsT, rhs):
                r = slice(half * H, (half + 1) * H)
                nc.tensor.matmul(
                    ps[r, :], lhsT=lhsT, rhs=rhs,
                    start=(mm_done[half] == 0),
                    stop=(mm_done[half] == 2 * nchh - 1))
                mm_done[half] += 1

            # hi/lo alternate in chunk pairs; chunk parity alternates the
            # PE column tile every instruction
            for c in range(0, nch2, 2):
                for cc in (c, c + 1):
                    mg, mglo, j = m_map[cc]
                    mm(cc % 2, mg[:, j, :], chi_map[cc][0][:, chi_map[cc][1], :])
                for cc in (c, c + 1):
                    mg, mglo, j = m_map[cc]
                    mm(cc % 2, mglo[:, j, :], clo_map[cc][0][:, clo_map[cc][1], :])

            # ---- epilogue ----
            stats = small.tile([P, 6], F32)
            nc.vector.bn_stats(out=stats, in_=ps)
            mv = small.tile([P, 2], F32)
            nc.vector.bn_aggr(out=mv, in_=stats)

            # rstd = 1/sqrt(var+eps) via bit-trick seed + 2 Newton steps
            # (pure DVE; keeps Sqrt's activation table out of the program).
            # In uniform mode we iterate on u = (var+eps)/g0^2 so the chain
            # directly yields sc = g0*rstd with no extra scaling op.
            ga = 1.0 / (gamma0 * gamma0) if uniform else 1.0
            ve = small.tile([P, 1], F32)
            nc.vector.tensor_scalar(out=ve, in0=mv[:, 1:2], scalar1=ga,
                                    scalar2=LN_EPS * ga, op0=Alu.mult,
                                    op1=Alu.add)
            ve2 = small.tile([P, 1], F32)   # -u/2
            nc.vector.tensor_scalar(out=ve2, in0=mv[:, 1:2], scalar1=-0.5 * ga,
                                    scalar2=-0.5 * LN_EPS * ga, op0=Alu.mult,
                                    op1=Alu.add)
            shi = small.tile([P, 1], I32)
            nc.vector.tensor_scalar(out=shi, in0=ve.bitcast(I32), scalar1=1,
                                    scalar2=None, op0=Alu.logical_shift_right)
            seedi = small.tile([P, 1], I32)
            nc.vector.tensor_scalar(out=seedi, in0=shi, scalar1=-1,
                                    scalar2=RSQRT_MAGIC, op0=Alu.mult,
                                    op1=Alu.add)
            x = seedi.bitcast(F32)
            for it in range(2):
                x2 = small.tile([P, 1], F32, tag=f"nx2_{it}")
                nc.vector.tensor_mul(x2, x, x)
                w = small.tile([P, 1], F32, tag=f"nw_{it}")
                nc.vector.tensor_scalar(out=w, in0=x2, scalar1=ve2,
                                        scalar2=1.5, op0=Alu.mult,
                                        op1=Alu.add)
                xn = small.tile([P, 1], F32, tag=f"nx_{it}")
                nc.vector.tensor_mul(xn, x, w)
                x = xn

            e = small.tile([P, T], F32)
            den = small.tile([P, 1], F32)
            if uniform:
                # softmax((ps - mean)*rstd*g0 - max(...)) == softmax((ps -
                # max ps)*rstd*g0): the mean cancels, and sc = rstd*g0 (the
                # Newton result) becomes the Exp scale read straight from PSUM
                negmx = small.tile([P, 1], F32)
                nc.vector.tensor_reduce(out=negmx, in_=ps, axis=Ax.X,
                                        op=Alu.max, negate=True)
                bias = small.tile([P, 1], F32)
                nc.vector.tensor_mul(bias, negmx, x)
                nc.scalar.activation(out=e, in_=ps, func=Act.Exp,
                                     bias=bias, scale=x, accum_out=den)
            else:
                z1 = small.tile([P, T], F32)
                nc.vector.tensor_scalar(out=z1, in0=ps, scalar1=mv[:, 0:1],
                                        scalar2=x, op0=Alu.subtract,
                                        op1=Alu.mult)
                z2 = small.tile([P, T], F32)
                nc.vector.tensor_mul(z2, z1, gb_t[:, 0:T])
                z3 = small.tile([P, T], F32)
                nc.vector.tensor_add(z3, z2, gb_t[:, T : 2 * T])
                negmx = small.tile([P, 1], F32)
                nc.vector.tensor_reduce(out=negmx, in_=z3, axis=Ax.X,
                                        op=Alu.max, negate=True)
                nc.scalar.activation(out=e, in_=z3, func=Act.Exp,
                                     bias=negmx, scale=1.0, accum_out=den)

            rden = small.tile([P, 1], F32)
            nc.vector.reciprocal(out=rden, in_=den)
            fac = small.tile([P, 1], F32)
            nc.vector.tensor_mul(fac, scal2, rden)
            fin = small.tile([P, T], F32)
            nc.vector.tensor_scalar_mul(fin, e, fac)
            nc.sync.dma_start(out=outd[:, :], in_=fin)
    nc.compile()
    return nc


def _get_nc(key=None):
    if key is None:
        key = _NC_CACHE["last_key"]
    if key not in _NC_CACHE:
        _NC_CACHE[key] = _build_nc(*key)
    return _NC_CACHE[key]


def _to_bf16_bits(x):
    # round-to-nearest-even bf16 via uint bit trick (ml_dtypes astype is
    # far too slow for MB-scale arrays)
    u = x.view(np.uint32)
    rounded = u + 0x7FFF + ((u >> 16) & 1)
    return (rounded >> 16).astype(np.uint16)


def _to_e4m3(x):
    # fast fp8e4m3 RNE for |x| < 448, with subnormals
    u = x.view(np.uint32)
    s = ((u >> 24) & 0x80).astype(np.uint32)
    mag = u & 0x7FFFFFFF
    r = mag + 0x7FFFF + ((mag >> 20) & 1)
    exp = (r >> 23).astype(np.int32) - 120      # e4m3-biased exponent
    man = (r >> 20) & 0x7
    # subnormal path: round(|x| * 2^9) gives the denormal bits directly
    man_d = np.rint(np.abs(x) * 512.0).astype(np.uint32)
    out = np.where(exp >= 1, (exp.astype(np.uint32) << 3) | man, man_d)
    return (s | out).astype(np.uint8)


def _make_in_maps(source_spikes, W_dyn, ln_gamma, ln_beta, gates, biases):
    source_spikes = np.asarray(source_spikes, dtype=np.float32)
    W_dyn = np.asarray(W_dyn, dtype=np.float32)
    ln_gamma = np.asarray(ln_gamma, dtype=np.float32)
    ln_beta = np.asarray(ln_beta, dtype=np.float32)
    gates = np.asarray(gates, dtype=np.float32)
    biases = np.asarray(biases, dtype=np.float32)

    # unfold (matches reference._unfold with kernel=stride=16)
    sp_unf = (
        source_spikes.reshape(B, PH, PATCH, PH, PATCH)
        .transpose(0, 1, 3, 2, 4)
        .reshape(B, N, S)
    )
    sp_unf = np.ascontiguousarray(sp_unf)

    # active-column index lists per core (patch-major order), split at the
    # patch-64 boundary; both halves pad to a common chunk count
    cores = []
    nchh = 1
    for c in range(NCORES):
        b, h = divmod(c, NCORES // B)
        n0 = h * P
        spv = np.ascontiguousarray(sp_unf[b, n0 : n0 + P])
        pid_arr, s_arr = np.nonzero(spv)
        ka = int(np.searchsorted(pid_arr, H))
        cores.append((b, n0, spv, pid_arr, s_arr, ka))
        nchh = max(nchh, -(-ka // P), -(-(len(pid_arr) - ka) // P))
    assert nchh <= MAX_NCH, f"active-column overflow: {nchh} chunks > {MAX_NCH}"
    nch2 = 2 * nchh

    uniform = bool(np.all(ln_gamma == ln_gamma[0]) and ln_gamma[0] > 0
                   and np.all(ln_beta == 0.0))
    gamma0 = float(ln_gamma[0] / TEMP)
    _NC_CACHE["last_key"] = (nchh, gamma0, uniform)

    iot_row = np.arange(P, dtype=np.float32).astype(NP_BF16)

    in_maps = []
    for b, n0, spv, pid_arr, s_arr, ka in cores:
        k = len(pid_arr)
        # gather active columns W_dyn[b, n0+pid, :, s] -> [k, T]
        cols = W_dyn[b, n0 : n0 + P][pid_arr, :, s_arr]
        hi_bits = _to_bf16_bits(cols)
        hi_f32 = (hi_bits.astype(np.uint32) << 16).view(np.float32)
        lo_bits = _to_e4m3((cols - hi_f32) * float(2 ** LOSH))

        # interleave the halves: even chunks = patches 0-63, odd = 64-127
        hi_pad = np.zeros((nch2, P, T), dtype=np.uint16)
        lo_pad = np.zeros((nch2, P, T), dtype=np.uint8)
        pid_pad = np.full((nch2, P), -1.0, dtype=np.float32)

        def fill(dst_h, dst_l, dst_p, bits_h, bits_l, pids, parity):
            # half `parity` occupies chunks parity, parity+2, ... slot-major
            kk = bits_h.shape[0]
            full, rem = divmod(kk, P)
            if full:
                sl = slice(parity, parity + 2 * full, 2)
                dst_h[sl] = bits_h[: full * P].reshape(full, P, T)
                dst_l[sl] = bits_l[: full * P].reshape(full, P, T)
                dst_p[sl] = pids[: full * P].reshape(full, P)
            if rem:
                ci = parity + 2 * full
                dst_h[ci, :rem] = bits_h[full * P :]
                dst_l[ci, :rem] = bits_l[full * P :]
                dst_p[ci, :rem] = pids[full * P :]

        fill(hi_pad, lo_pad, pid_pad, hi_bits[:ka], lo_bits[:ka],
             pid_arr[:ka], 0)
        fill(hi_pad, lo_pad, pid_pad, hi_bits[ka:], lo_bits[ka:],
             pid_arr[ka:], 1)

        def pack(flat):
            return np.ascontiguousarray(
                flat.transpose(1, 0, 2).reshape(P, nch2 * T))

        meta = np.empty((P, nch2 + P), dtype=NP_BF16)
        meta[:, 0:nch2] = pid_pad.T.astype(NP_BF16)
        meta[:, nch2:] = iot_row[None, :]

        prm = np.empty((P, 2), dtype=np.float32)
        prm[:, 0] = gates[n0 : n0 + P]
        prm[:, 1] = biases[n0 : n0 + P]

        im = {
            "chi": pack(hi_pad).view(NP_BF16),
            "clo": pack(lo_pad).view(NP_FP8),
            "meta": meta,
            "sp": spv.astype(NP_BF16),
            "prm": prm,
        }
        if not uniform:
            gb = np.empty((P, 2 * T), dtype=np.float32)
            gb[:, 0:T] = ln_gamma / TEMP
            gb[:, T : 2 * T] = ln_beta / TEMP
            im["gb"] = gb
        in_maps.append(im)
    return in_maps


def _assemble(results):
    out_bnt = np.empty((B, N, T), dtype=np.float32)
    for c in range(NCORES):
        b, h = divmod(c, NCORES // B)
        n0 = h * P
        out_bnt[b, n0 : n0 + P] = results[c]["out"]
    # fold (matches reference._fold)
    return np.ascontiguousarray(
        out_bnt.reshape(B, PH, PH, PATCH, PATCH)
        .transpose(0, 1, 3, 2, 4)
        .reshape(B, GRID, GRID)
    )


def run_sharded(inputs: dict, trace: bool = False):
    """Run the SPMD bass kernel on 8 cores. Returns (output, BassKernelResults)."""
    in_maps = _make_in_maps(**inputs)
    nc = _get_nc()
    res = bass_utils.run_bass_kernel_spmd(nc, in_maps, list(range(NCORES)),
                                          trace=trace)
    return _assemble(res.results), res


def kernel(**inputs) -> np.ndarray:
    out, _ = run_sharded(inputs, trace=False)
    return out
